# revision 9
# baseline (speedup 1.0000x reference)
"""Segment-sum (jax.ops.segment_sum(H, X_node, num_segments=V)) on 8 trn2
NeuronCores.

Strategy
--------
Host-side sharding (sorted deal): segments sorted by token count are
dealt in groups of 128 similar-count segments to (core, block, slot);
each core produces a disjoint set of output rows, so no device reduce is
needed and the host applies the inverse permutation on gather.

Because slot counts within a block are nearly uniform, ~98% of tiles are
identity-diagonals: the r-th token of slot s sits at partition s of tile
r, and the tensor engine accumulates it with a CONSTANT identity lhsT —
no per-tile one-hot build.  Only per-slot overflow tokens go to dense
tiles whose one-hot is built on the vector engine (tensor_scalar
is_equal).  Per-local-block (J_diag, T_overflow) budgets are shared by
all 8 cores (one static program).

H is shipped as plain bf16 (2B/elem — half the f32 bytes; l2 rel err
~1.1e-3, well under the 2e-2 gate).  To keep the tensor engine off the
critical path at the narrower 64-col moving tensor, consecutive diagonal
tiles are PAIRED into one 128-wide matmul sharing the constant identity
lhsT; block layouts are padded to even slot counts so pairs never
straddle DMA chunks.  At flush the two PSUM halves are added on the
vector engine into the output stage, which is streamed out in quarters.

Cost model: ~100us/core at the modeled 358GB/s DMA floor for the
~33MB/core streamed (PE ~54us, DVE ~38us, both overlapped).
"""

import math
import sys

sys.path.insert(0, "/opt/trn_rl_repo")

import numpy as np
import ml_dtypes

import concourse.bacc as bacc
import concourse.mybir as mybir
import concourse.tile as tile
from concourse.bass_utils import run_bass_kernel_spmd

P = 128          # partitions / tokens per tile / segments per block
D = 64           # feature dim
V = 100000       # number of segments
NCORES = 8
NB = 98          # blocks per core (8*98*128 = 100352 >= V)
SLICE = NB * P   # segments per core
TC = 64          # tiles per DMA chunk (must be even: pairs are 2 slots)

_BUILD_CACHE: dict = {}

HP_BUFS = 4    # H chunk buffers
OP_BUFS = 8    # onehot tiles in flight (DVE -> PE pipeline depth)


def _block_layout(J: int, T: int):
    """Instruction/slot layout for one block, shared by host and device.

    Returns (slots, instrs) where instrs is a list of
    ("pair", slot) — 128-wide matmul over slots (slot, slot+1), identity
                     lhsT; second slot may be zero padding.
    ("ovf", slot, width) — overflow tile with DVE-built one-hot lhsT;
                     width 2 only when J == 0 (so the block's first
                     matmul still initializes the full PSUM width).
    Slot count is always even so pairs never straddle an even-sized
    chunk boundary."""
    instrs = []
    s = 0
    for _ in range(math.ceil(J / 2)):
        instrs.append(("pair", s))
        s += 2
    for t in range(T):
        if J == 0 and t == 0:
            instrs.append(("ovf", s, 2))
            s += 2
        else:
            instrs.append(("ovf", s, 1))
            s += 1
    if s % 2:
        s += 1
    return s, instrs


def _build(nb: int, budgets: tuple, tc: int, nchunks: int,
           variant: str = "full"):
    """Static SPMD program. budgets: per-local-block (J_diag, T_overflow)
    tuples, identical across cores; diagonal tiles are consumed in pairs
    by 128-wide matmuls with a constant identity lhsT; overflow tiles
    build a one-hot on DVE."""
    key = (nb, budgets, tc, nchunks, variant, HP_BUFS, OP_BUFS)
    if key in _BUILD_CACHE:
        return _BUILD_CACHE[key]
    reps = 1
    small_out = False
    if variant.startswith("ts"):
        reps = int(variant[2:])
        small_out = True
        variant = "full"
    # flat instruction schedule: (block, kind, slot, width, first, last)
    sched = {}  # first slot -> instruction
    off = 0
    for lb, (J, T) in enumerate(budgets):
        slots, instrs = _block_layout(J, T)
        for i, ins in enumerate(instrs):
            if ins[0] == "pair":
                kind, s = ins
                w = 2
            else:
                kind, s, w = ins
            sched[off + s] = (lb, kind, w, i == 0, i == len(instrs) - 1)
        off += slots
    ntiles = off
    assert nchunks * tc >= ntiles
    ew = D  # bf16 elems per token per slot
    nc = bacc.Bacc("TRN2")
    hin = nc.dram_tensor("h", [nchunks, P, tc * ew], mybir.dt.bfloat16,
                         kind="ExternalInput")
    lin = nc.dram_tensor("lo", [P, nchunks * tc], mybir.dt.float32,
                         kind="ExternalInput")
    iin = nc.dram_tensor("iota", [P, P], mybir.dt.bfloat16,
                         kind="ExternalInput")
    idn = nc.dram_tensor("ident", [P, P], mybir.dt.bfloat16,
                         kind="ExternalInput")
    out = nc.dram_tensor("out", [P, D if small_out else nb * D],
                         mybir.dt.float32, kind="ExternalOutput")

    with tile.TileContext(nc) as tc_ctx:
        with (
            tc_ctx.tile_pool(name="hp", bufs=HP_BUFS) as hp,
            tc_ctx.tile_pool(name="op", bufs=OP_BUFS) as op,
            tc_ctx.tile_pool(name="pp", bufs=8, space="PSUM") as pp,
            tc_ctx.tile_pool(name="cp", bufs=1) as cp,
        ):
            iota = cp.tile([P, P], mybir.dt.bfloat16, tag="iota")
            nc.sync.dma_start(iota[:], iin[:])
            const_oh = cp.tile([P, P], mybir.dt.bfloat16, tag="constoh")
            nc.sync.dma_start(const_oh[:], idn[:])
            ostage = cp.tile([P, nb * D], mybir.dt.float32, tag="ostage")
            if variant in ("dmaonly", "nope"):
                nc.gpsimd.memset(ostage[:], 0.0)
            lall = cp.tile([P, nchunks * tc], mybir.dt.float32, tag="lall")
            nc.sync.dma_start(lall[:], lin[:])
            psum = None
            for _rep, ch in ((r, c) for r in range(reps)
                             for c in range(nchunks)):
                # last chunk: stream only the valid tiles (skip tail padding)
                nt = min(tc, ntiles - ch * tc)
                htile = hp.tile([P, tc * ew], mybir.dt.bfloat16)
                nc.sync.dma_start(htile[:, :nt * ew], hin[ch, :, :nt * ew])
                if variant == "dmaonly":
                    continue
                for k in range(nt):
                    g = ch * tc + k
                    ins = sched.get(g)
                    if ins is None:
                        continue
                    b, kind, w, first, last = ins
                    ohtile = None
                    if variant != "nodve" and kind == "ovf":
                        ohtile = op.tile([P, P], mybir.dt.bfloat16)
                        nc.vector.tensor_scalar(
                            out=ohtile[:],
                            in0=iota[:],
                            scalar1=lall[:, g:g + 1],
                            scalar2=None,
                            op0=mybir.AluOpType.is_equal,
                        )
                    if variant == "nope":
                        continue
                    oh = (const_oh[:] if (variant == "nodve" or kind != "ovf")
                          else ohtile[:])
                    if first:
                        psum = pp.tile([P, 2 * ew], mybir.dt.float32)
                    nc.tensor.matmul(
                        psum[:, :w * ew],
                        lhsT=oh,
                        rhs=htile[:, k * ew:(k + w) * ew],
                        start=first,
                        stop=last,
                    )
                    if last:
                        # DVE may read only one PSUM operand per instruction
                        nc.vector.tensor_copy(
                            out=ostage[:, b * D:(b + 1) * D],
                            in_=psum[:, :D],
                        )
                        nc.vector.tensor_add(
                            out=ostage[:, b * D:(b + 1) * D],
                            in0=ostage[:, b * D:(b + 1) * D],
                            in1=psum[:, D:2 * D],
                        )
                        # stream completed quarters of ostage out early so
                        # the final store overlaps compute
                        if not small_out and _rep == reps - 1:
                            q = nb // 4
                            if b + 1 in (q, 2 * q, 3 * q):
                                s = (b + 1 - q) * D
                                nc.sync.dma_start(
                                    out[:, s:(b + 1) * D],
                                    ostage[:, s:(b + 1) * D])
            if small_out:
                nc.sync.dma_start(out[:], ostage[:, :D])
            else:
                q = nb // 4
                nc.sync.dma_start(out[:, 3 * q * D:], ostage[:, 3 * q * D:])
    nc.finalize()
    _BUILD_CACHE[key] = nc
    return nc


def _host_prep(H: np.ndarray, seg: np.ndarray, ncores: int, nb: int,
               tc: int):
    """Sorted-deal sharding: segments sorted by count desc are dealt in
    groups of 128 similar-count segments to (core = g % ncores,
    local block = g // ncores, slot = position).  Within a block the slot
    counts are nearly uniform, so most tiles are identity-diagonals (the
    r-th token of slot s at partition s of tile r) needing no one-hot
    build; per-slot overflow beyond each block's diagonal depth J goes to
    dense one-hot tiles.  Per-local-block budgets (J, T_overflow) are
    shared by all cores (one static program)."""
    n, d = H.shape
    v = ncores * nb * P
    nblocks = ncores * nb
    cnt_seg = np.bincount(seg, minlength=v)
    order_seg = np.argsort(-cnt_seg, kind="stable")
    # seg order_seg[i]: group g = i // P, slot = i % P
    g_of = np.arange(v) // P
    blk_of = np.empty(v, np.int32)
    slot_of = np.empty(v, np.int32)
    blk_of[order_seg] = ((g_of % ncores) * nb + g_of // ncores).astype(np.int32)
    slot_of[order_seg] = (np.arange(v) % P).astype(np.int32)

    # per-(block, slot) counts and per-local-block budgets
    per_bs = np.zeros((nblocks, P), np.int64)
    per_bs[blk_of, slot_of] = cnt_seg
    budgets = []
    for lb in range(nb):
        M = per_bs[[c * nb + lb for c in range(ncores)]]
        best = None
        for J in range(int(M.min()), int(M.max()) + 1):
            ovf = int(np.maximum(M - J, 0).sum(axis=1).max())
            tov = -(-ovf // P)
            # <= : on equal totals prefer larger J (fewer one-hot tiles)
            if best is None or J + tov <= best[0] + best[1]:
                best = (J, tov)
        budgets.append(best)
    budgets = tuple(budgets)
    # slot offsets per local block (even-padded layout)
    slots_of = [_block_layout(J, T)[0] for J, T in budgets]
    off = np.zeros(nb + 1, np.int64)
    np.cumsum(np.array(slots_of), out=off[1:])
    ntiles = int(off[-1])
    nchunks = int(math.ceil(ntiles / tc))
    rows_pad = nchunks * tc * P

    # token destinations
    key = blk_of[seg].astype(np.int64) * P + slot_of[seg]
    order = np.argsort(key, kind="stable")
    skey = key[order]
    cnt_key = np.bincount(skey, minlength=nblocks * P)
    starts_key = np.zeros(nblocks * P + 1, np.int64)
    np.cumsum(cnt_key, out=starts_key[1:])
    r = np.arange(n) - starts_key[skey]            # rank within (block, slot)
    sblk = skey // P
    cnt_blk = np.bincount(sblk, minlength=nblocks)
    starts_blk = np.zeros(nblocks + 1, np.int64)
    np.cumsum(cnt_blk, out=starts_blk[1:])
    lbs = sblk % nb
    core = sblk // nb
    J_of = np.array([b[0] for b in budgets], np.int64)[lbs]
    is_ovf = r >= J_of
    co = np.cumsum(is_ovf)
    coz = np.concatenate([[0], co])[starts_blk[sblk]]  # ovf before block
    q = co - 1 - coz                                   # ovf index in block
    # overflow slot base and per-tile slot within the block
    soff_of = np.array([2 * math.ceil(b[0] / 2) for b in budgets],
                       np.int64)[lbs]
    ovf_pad = np.array([1 if b[0] == 0 and b[1] > 0 else 0 for b in budgets],
                       np.int64)[lbs]  # J==0: pad slot after first ovf tile
    ovf_t = q // P
    ovf_slot = soff_of + np.where(ovf_t == 0, 0, ovf_t + ovf_pad)
    tile_idx = np.where(is_ovf, off[lbs] + ovf_slot, off[lbs] + r)
    part = np.where(is_ovf, q % P, skey % P)
    dstrow = tile_idx * P + part
    lo_val = (skey % P).astype(np.float32)

    iota = np.broadcast_to(np.arange(P), (P, P)).astype(ml_dtypes.bfloat16)
    ident = np.eye(P, dtype=ml_dtypes.bfloat16)
    in_maps = []
    for c in range(ncores):
        sel = core == c
        dst_c = dstrow[sel]
        assert dst_c.max() < rows_pad
        assert np.unique(dst_c).size == dst_c.size, "dst collision"
        rows = np.zeros((rows_pad, d), ml_dtypes.bfloat16)
        lo = np.zeros(rows_pad, np.float32)
        rows[dst_c] = H[order[sel]].astype(ml_dtypes.bfloat16)
        lo[dst_c] = lo_val[sel]
        hl = rows.reshape(rows_pad // P, P, d)
        hdev = np.ascontiguousarray(
            hl.reshape(nchunks, tc, P, d).transpose(0, 2, 1, 3)
        ).reshape(nchunks, P, tc * d)
        lodev = np.ascontiguousarray(lo.reshape(nchunks * tc, P).T)
        in_maps.append({"h": hdev, "lo": lodev, "iota": iota,
                        "ident": ident})
    outperm = blk_of.astype(np.int64) * P + slot_of  # seg -> output slot
    return in_maps, budgets, nchunks, outperm


def _unshard(results, ncores: int, nb: int, outperm: np.ndarray) -> np.ndarray:
    full = np.empty((ncores * nb * P, D), np.float32)
    for c in range(ncores):
        o = np.asarray(results[c]["out"]).reshape(P, nb, D)
        full[c * nb * P:(c + 1) * nb * P] = (
            o.transpose(1, 0, 2).reshape(nb * P, D)
        )
    return full[outperm]


def _run(H, X_node, trace=False, trace_kwargs=None):
    H = np.ascontiguousarray(np.asarray(H, dtype=np.float32))
    seg = np.asarray(X_node).astype(np.int64)
    in_maps, budgets, nchunks, outperm = _host_prep(H, seg, NCORES, NB, TC)
    nc = _build(NB, budgets, TC, nchunks)
    kwargs = {}
    if trace:
        kwargs = dict(trace=True, trace_cores=list(range(NCORES)),
                      stitch_traces=False)
        if trace_kwargs:
            kwargs.update(trace_kwargs)
    res = run_bass_kernel_spmd(nc, in_maps, core_ids=list(range(NCORES)),
                               **kwargs)
    out = _unshard(res.results, NCORES, NB, outperm[:V])
    return out, res


def kernel(H, X_node) -> np.ndarray:
    out, _ = _run(H, X_node, trace=False)
    return out


if __name__ == "__main__":
    # tiny smoke test on hardware (all 8 cores, small V')
    rng = np.random.default_rng(0)
    n_small, v_small, nb_small, tc_small = 6000, NCORES * 2 * P, 2, 4
    Hs = rng.standard_normal((n_small, D)).astype(np.float32)
    segs = rng.integers(0, v_small, size=n_small).astype(np.int64)
    in_maps, budgets, nchunks, outperm = _host_prep(Hs, segs, NCORES, nb_small,
                                                    tc_small)
    nc = _build(nb_small, budgets, tc_small, nchunks)
    res = run_bass_kernel_spmd(nc, in_maps, core_ids=list(range(NCORES)))
    got = _unshard(res.results, NCORES, nb_small, outperm[:v_small])
    exp = np.zeros((v_small, D), np.float32)
    np.add.at(exp, segs, Hs)
    err = np.abs(got - exp).max() / max(1e-9, np.abs(exp).max())
    print(f"smoke: ntiles={sum(_block_layout(j, t)[0] for j, t in budgets)} "
          f"nchunks={nchunks} max-rel-err={err:.3e}")
    assert err < 5e-3, "smoke test failed"
    print("SMOKE PASS")
